# revision 24
# baseline (speedup 1.0000x reference)
"""Single-head causal attention (B=16, S=2048, d_model=384, d_q=64) on 8 trn2 cores.

Sharding: data-parallel over batch -- 2 batches per core.

v2 design (bf16 matmul inputs, fp32 PSUM accumulation):
  - Host marshaling: x is passed to each core pre-transposed as bf16
    [BPC, 3, 128, S] (d-major chunks), so xT loads straight into SBUF with
    plain linear DMAs -- no on-chip transpose or cast (saves ~20% of PE work
    and ~half the DVE work vs computing xT on-chip).  Wq|Wk are host-packed
    into a single [3, 128, 128] bf16 stationary (Q in cols 0-63, K in
    64-127); Wv is [3, 128, 64] bf16.
  - Projections: per 512-col chunk, 3 accumulating matmuls with the packed
    wqk stationary produce Q^T and K^T stacked in PSUM; two DVE copies
    split them into qt/kt [64, S] bf16.  V is computed *naturally* [s, 64]
    with lhsT = xT chunk (stationary) and rhs = Wv chunk, augmented with a
    ones column -> V_aug [128, 65] per key block (the ones column makes the
    softmax denominator fall out of the PV matmul for free).
  - Attention in *transposed* score layout: scoresT[k, q] =
    matmul(lhsT=K^T block [64, 128], rhs=Q^T [64, q]).  exp() runs on the
    scalar engine straight from PSUM with the 1/8 scale folded in; no
    max-subtraction needed (scores are O(+-10)).  To amortize the ~293ns
    fixed cost per ACT instruction, small tail blocks are *paired* so each
    activation covers a full [128, 1024] PSUM tile: per panel the blocks
    with widths (896,128), (768,256), (640,384) share one PSUM tile and one
    exp() instruction (18 ACTs per batch instead of 24).
  - Causal masking: score matmuls only cover q >= 128*i; the diagonal
    128x128 block is masked after exp by zeroing q < k with
    gpsimd.affine_select (gpsimd is otherwise idle).
  - out^T_aug[e', q] accumulates V_aug^T @ P over key blocks in a PSUM acc
    tile per 1024-wide panel; row 64 is the softmax denominator.
  - Epilogue: out^T_aug is PE-transposed back in 128-col blocks (4 share a
    PSUM bank; start=True only clears has_written bits, data of earlier
    transposes survives), divided by the denominator via per-partition
    reciprocal + tensor_scalar_mul, and DMA'd out one panel at a time.
  - Emission order is hand-pipelined for the in-order engine queues:
    scores(u+1) is emitted before PV(u), batch 1's phase-A steps are
    interleaved into batch 0's attention, and batch 1's first attention
    units are interleaved into batch 0's final epilogue so the PE never
    sees a multi-us idle window (HAM clock-gate stays at 8/8).
"""

import numpy as np

B, S, D, E = 16, 2048, 384, 64
N_CORES = 8
BPC = B // N_CORES  # batches per core
NB = S // 128  # 16 key blocks of 128
H = 1024  # attention column-panel width
SCALE = 1.0 / 8.0  # 1/sqrt(d_q)

_cache = {}


def _pieces(lo, hi):
    """Split [lo, hi) at 512 boundaries (PSUM bank granularity)."""
    out = []
    a = lo
    while a < hi:
        b_ = min((a // 512 + 1) * 512, hi)
        out.append((a, b_))
        a = b_
    return out


def _split_multi_waits(nc, max_waits=1):
    """Walrus codegen on this image rejects instructions carrying more than
    one sync wait (setupSyncWait: 'Too many sync wait commands').  Engines
    execute their queue in order, so excess waits can be moved onto NOP
    instructions inserted immediately before the owning instruction."""
    import concourse.mybir as mybir

    k = 0
    for f in nc.m.functions:
        for bb in f.blocks:
            insts = bb.instructions
            out = []
            changed = False
            for ins in insts:
                si = getattr(ins, "sync_info", None)
                waits = list(si.on_wait) if si is not None else []
                if len(waits) > max_waits:
                    changed = True
                    for extra in waits[:-max_waits]:
                        nop = mybir.InstNoOp(
                            name=f"wsplit-{k}", ins=[], outs=[]
                        )
                        k += 1
                        nop.engine = ins.engine
                        nop.sync_info = mybir.SyncInfo(
                            on_wait=[extra], on_update=[]
                        )
                        out.append(nop)
                    ins.sync_info = mybir.SyncInfo(
                        on_wait=waits[-max_waits:],
                        on_update=list(si.on_update),
                    )
                out.append(ins)
            if changed:
                bb.instructions = out


def _install_patches():
    """Register the NTFF profile hook so trace=True works under axon."""
    import sys
    import types

    if "antenv.axon_hooks" not in sys.modules:
        mod = types.ModuleType("antenv.axon_hooks")
        state = {"hook": None}
        mod.set_axon_ntff_profile_hook = lambda h: state.__setitem__("hook", h)
        mod.get_axon_ntff_profile_hook = lambda: state["hook"]
        sys.modules["antenv.axon_hooks"] = mod
        try:
            import antenv

            antenv.axon_hooks = mod
            if "/root/.axon_site" not in sys.path:
                sys.path.insert(0, "/root/.axon_site")
            from trn_agent_boot.trn_boot import _ntff_profile_via_ctypes

            mod.set_axon_ntff_profile_hook(
                _ntff_profile_via_ctypes("/opt/axon/libaxon_pjrt.so")
            )
        except Exception:
            pass
    import concourse.bass_utils as bu

    bu.upload_artifacts = lambda tmpdir: tmpdir


def _build_nc():
    import concourse.bass as bass
    import concourse.mybir as mybir
    from concourse.bass import ts
    from concourse.masks import make_identity
    from concourse.tile import TileContext

    f32 = mybir.dt.float32
    bf16 = mybir.dt.bfloat16
    Exp = mybir.ActivationFunctionType.Exp

    nc = bass.Bass()
    x_d = nc.dram_tensor("x", [BPC, 3, 128, S], bf16, kind="ExternalInput")
    wqk_d = nc.dram_tensor("wqk", [3, 128, 128], bf16, kind="ExternalInput")
    wv_d = nc.dram_tensor("wv", [3, 128, E], bf16, kind="ExternalInput")
    out_d = nc.dram_tensor("out", [BPC, S, E], f32, kind="ExternalOutput")

    with TileContext(nc) as tc:
        with (
            tc.tile_pool(name="consts", bufs=1) as cpool,
            tc.tile_pool(name="xt", bufs=2) as xtpool,
            tc.tile_pool(name="qt", bufs=2) as qtpool,
            tc.tile_pool(name="kt", bufs=2) as ktpool,
            tc.tile_pool(name="vaug", bufs=2) as vpool,
            tc.tile_pool(name="pt", bufs=3) as ptpool,
            tc.tile_pool(name="ott", bufs=2) as otpool,
            tc.tile_pool(name="oo", bufs=2) as opool,
            tc.tile_pool(name="rc", bufs=2) as rcpool,
            tc.tile_pool(name="ps", bufs=2, space="PSUM") as pspool,
            tc.tile_pool(name="pss", bufs=2, space="PSUM") as pshalf,
            tc.tile_pool(name="acc", bufs=1, space="PSUM") as accpool,
        ):
            ident = cpool.tile([128, 128], f32, tag="ident")
            make_identity(nc, ident[:])
            identb = cpool.tile([128, 128], bf16, tag="identb")
            make_identity(nc, identb[:])

            wqk_sb = cpool.tile([128, 3 * 128], bf16, tag="wqk")
            wv_sb = cpool.tile([128, 3 * E], bf16, tag="wv")

            def load_weights():
                # one combined DMA per weight tensor (each dma_start costs
                # ~650ns of issue time on its engine queue -- keep count low)
                nc.sync.dma_start(
                    wqk_sb[:].rearrange("p (c j) -> p c j", c=3),
                    wqk_d[:, :, :].rearrange("c p j -> p c j"),
                )
                nc.sync.dma_start(
                    wv_sb[:].rearrange("p (c j) -> p c j", c=3),
                    wv_d[:, :, :].rearrange("c p j -> p c j"),
                )
                # Warm the ACT exp table-set (~2.7us load) while phase A
                # runs -- emitted after scalar's dma_start so the x DMA
                # issues first on that queue.
                warm = cpool.tile([1, 8], f32, tag="warm")
                nc.scalar.activation(warm[:], ident[:1, 0:8], Exp)

            wdum = cpool.tile([128, 640], bf16, tag="wdum")

            def pe_warmup():
                # ~20 back-to-back dummy N=512 matmuls (~5-6us wall) while
                # the x DMAs are in flight: trips the HAM activity monitor
                # to K=8/8 and keeps the PE busy until data arrives, so the
                # real work starts at full clock with no >3.4us idle window
                # (which would re-throttle).  The dummy operand is memset on
                # the DVE so the warmup doesn't wait for the gpsimd iotas.
                wps = pshalf.tile([128, H], f32, tag="pss", name="warmps")
                for _ in range(30):
                    nc.tensor.matmul(
                        wps[:, 0:128], identb[:], identb[:],
                        start=True, stop=True,
                    )
                return wps

            state = {}

            def phase_a(b):
                """x load, QK projection, V+ones.  Yields after each
                PSUM-consuming step so it can be interleaved into the
                previous batch's attention emission."""
                st = state[b] = {}
                xt_all = xtpool.tile(
                    [128, 3 * S], bf16, tag="xt", name=f"xt_{b}"
                )
                xt3 = xt_all[:].rearrange("p (c s) -> p c s", c=3)
                # dma_starts on one engine queue serialize (~650ns issue +
                # sequential ring drain), and both HWDGE queues share the 16
                # SDMA engines (~414 GB/s aggregate).  For batch 0, issue
                # 1024-col halves across the sync+scalar queues interleaved
                # with the (tiny but prerequisite) weight DMAs, so the first
                # projection chunk is runnable ~10us in; the PE warmup
                # bridges the gap at full-ish occupancy.  Batch 1 avoids the
                # scalar queue (busy with exp()).
                engs = (
                    [nc.sync, nc.scalar, nc.gpsimd]
                    if b == 0
                    else [nc.sync, nc.gpsimd, nc.sync]
                )
                for c in range(3):
                    engs[c].dma_start(xt3[:, c, :], x_d[b, c])
                yield "pa"

                def xts(c, lo, width):
                    return xt_all[:, c * S + lo : c * S + lo + width]

                st["xts"] = xts
                qt = st["qt"] = qtpool.tile(
                    [64, S], bf16, tag="qt", name=f"qt_{b}"
                )
                kt = st["kt"] = ktpool.tile(
                    [64, S], bf16, tag="kt", name=f"kt_{b}"
                )
                for n in range(4):
                    pq = pspool.tile(
                        [128, 512], f32, tag="ps", name=f"pq_{b}_{n}"
                    )
                    for c in range(3):
                        nc.tensor.matmul(
                            pq[:],
                            wqk_sb[:, ts(c, 128)],
                            xts(c, 512 * n, 512),
                            start=(c == 0),
                            stop=(c == 2),
                        )
                    nc.vector.tensor_copy(qt[:, ts(n, 512)], pq[0:64, :])
                    nc.vector.tensor_copy(kt[:, ts(n, 512)], pq[64:128, :])
                    yield "pa"

                va_all = st["va"] = vpool.tile(
                    [128, NB * (E + 1)], bf16, tag="va", name=f"va_{b}"
                )
                va3 = va_all[:].rearrange("p (k e) -> p k e", k=NB)
                for g in range(2):
                    pv = pspool.tile(
                        [128, 512], f32, tag="ps", name=f"pv_{b}_{g}"
                    )
                    for j in range(8):
                        k = 8 * g + j
                        for c in range(3):
                            nc.tensor.matmul(
                                pv[:, ts(j, E)],
                                xts(c, 128 * k, 128),
                                wv_sb[:, ts(c, E)],
                                start=(c == 0),
                                stop=(c == 2),
                            )
                    nc.vector.tensor_copy(
                        va3[:, 8 * g : 8 * g + 8, 0:E],
                        pv[:].rearrange("p (k e) -> p k e", k=8),
                    )
                    yield "pa"
                nc.gpsimd.memset(va3[:, :, E : E + 1], 1.0)

            def attention(b):
                """Panel attention + epilogue.  Yields ('u',) per unit and
                ('ep',) during the epilogue (tail-overlap hook)."""
                st = state[b]
                qt, kt, va_all = st["qt"], st["kt"], st["va"]
                for h in range(2):
                    base = H * h
                    nfull = base // 128 + 1
                    t0 = base // 128 + 1
                    # units: list of [(block, off)] sharing one PSUM tile
                    units = [[(i, 0)] for i in range(nfull)]
                    units += [
                        [(t0, 0), (t0 + 6, 896)],
                        [(t0 + 1, 0), (t0 + 5, 768)],
                        [(t0 + 2, 0), (t0 + 4, 640)],
                        [(t0 + 3, 0)],
                    ]

                    def qlo_of(i):
                        return max(128 * i, base)

                    # Precompute PV start/stop flags: first/last emitted
                    # matmul per acc bank (emission follows unit order).
                    pv_seq = []
                    for u in units:
                        for (i, off) in u:
                            qlo = qlo_of(i)
                            for (a, b_) in _pieces(qlo - base, H):
                                pv_seq.append((i, a, b_))
                    first_in_bank = {}
                    last_in_bank = {}
                    for idx, (i, a, b_) in enumerate(pv_seq):
                        bank = a // 512
                        first_in_bank.setdefault(bank, idx)
                        last_in_bank[bank] = idx
                    pv_flags = {}
                    for idx, (i, a, b_) in enumerate(pv_seq):
                        bank = a // 512
                        pv_flags[idx] = (
                            first_in_bank[bank] == idx,
                            last_in_bank[bank] == idx,
                        )

                    acc = accpool.tile(
                        [E + 1, H], f32, tag="acc", name=f"acc_{b}_{h}"
                    )
                    pv_idx = [0]


                    def emit_pv(unit, pt, acc=acc, base=base):
                        for (i, off) in unit:
                            qlo = qlo_of(i)
                            for (a, b_) in _pieces(qlo - base, H):
                                sflag, eflag = pv_flags[pv_idx[0]]
                                pv_idx[0] += 1
                                po = off + a - (qlo - base)
                                nc.tensor.matmul(
                                    acc[:, a:b_],
                                    va_all[:, 65 * i : 65 * i + 65],
                                    pt[:, po : po + (b_ - a)],
                                    start=sflag,
                                    stop=eflag,
                                )

                    pending = None
                    for u in units:
                        wtot = sum(
                            base + H - qlo_of(i) for (i, _off) in u
                        )
                        ps_s = pshalf.tile(
                            [128, H], f32, tag="pss", name=f"ss_{b}_{h}"
                        )
                        # scores for each block of the unit; start/stop =
                        # first/last emitted matmul per bank of this tile
                        sc = []
                        for (i, off) in u:
                            qlo = qlo_of(i)
                            w = base + H - qlo
                            for (p0, p1) in _pieces(off, off + w):
                                sc.append((i, qlo, off, p0, p1))
                        sbank_first = {}
                        sbank_last = {}
                        for idx, (i, qlo, off, p0, p1) in enumerate(sc):
                            bank = p0 // 512
                            sbank_first.setdefault(bank, idx)
                            sbank_last[bank] = idx
                        for idx, (i, qlo, off, p0, p1) in enumerate(sc):
                            bank = p0 // 512
                            nc.tensor.matmul(
                                ps_s[:, p0:p1],
                                kt[:, ts(i, 128)],
                                qt[:, qlo + (p0 - off) : qlo + (p1 - off)],
                                start=(sbank_first[bank] == idx),
                                stop=(sbank_last[bank] == idx),
                            )
                        pt = ptpool.tile(
                            [128, H], bf16, tag="pt", name=f"pt_{b}_{h}"
                        )
                        nc.scalar.activation(
                            pt[:, :wtot], ps_s[:, :wtot], Exp, scale=SCALE
                        )
                        for (i, off) in u:
                            if 128 * i >= base:  # diagonal block: mask q<k
                                nc.gpsimd.affine_select(
                                    out=pt[:, off : off + 128],
                                    in_=pt[:, off : off + 128],
                                    compare_op=mybir.AluOpType.is_ge,
                                    fill=0.0,
                                    base=0,
                                    pattern=[[1, 128]],
                                    channel_multiplier=-1,
                                )
                        if pending is not None:
                            emit_pv(*pending)
                        pending = (u, pt)
                        yield ("u", h)
                    emit_pv(*pending)

                    # epilogue: transpose acc back, divide by denominator
                    ott = otpool.tile(
                        [E + 1, H], f32, tag="ott", name=f"ot_{b}_{h}"
                    )
                    nc.vector.tensor_copy(ott[:], acc[:])
                    oo = opool.tile(
                        [128, 8 * E], f32, tag="oo", name=f"oo_{b}_{h}"
                    )
                    for half in range(2):
                        pe_ = pspool.tile(
                            [128, 512], f32, tag="ps", name=f"pe_{b}_{h}"
                        )
                        pe3 = pe_[:, 0 : 4 * 65].rearrange(
                            "p (t e) -> p t e", t=4
                        )
                        for t4 in range(4):
                            tt = 4 * half + t4
                            nc.tensor.transpose(
                                pe_[:, 65 * t4 : 65 * t4 + 65],
                                ott[:, ts(tt, 128)],
                                ident[: E + 1, : E + 1],
                            )
                        rc = rcpool.tile(
                            [128, 4], f32, tag="rc", name=f"rc_{b}_{h}"
                        )
                        nc.vector.reciprocal(rc[:], pe3[:, :, E : E + 1])
                        for t4 in range(4):
                            tt = 4 * half + t4
                            nc.vector.tensor_scalar_mul(
                                oo[:, ts(tt, E)],
                                pe3[:, t4, 0:E],
                                rc[:, t4 : t4 + 1],
                            )
                        # DMA out per half so the store overlaps the other
                        # half's transposes instead of serializing the tail
                        nc.sync.dma_start(
                            out_d[
                                b, base + 512 * half : base + 512 * (half + 1), :
                            ].rearrange("(j p) e -> p j e", p=128),
                            oo[:, ts(half, 4 * E)].rearrange(
                                "p (j e) -> p j e", j=4
                            ),
                        )
                        yield ("ep", h)

            # Interleaved emission: batch 1's phase-A steps are alternated
            # with batch 0's attention units, and batch 1's first attention
            # units fill batch 0's epilogue, so the in-order PE queue never
            # idles long enough for the HAM governor to drop to half clock.
            pa0 = phase_a(0)
            next(pa0)  # x DMAs first: they gate everything downstream
            load_weights()
            pe_warmup()
            for _ in pa0:
                pass
            pa1 = phase_a(1)
            a0 = attention(0)
            a1 = None
            for tag in a0:
                if next(pa1, None) is not None:
                    continue
                # phase_a(1) exhausted: fill batch 0's final epilogue
                # with batch 1's first attention units (tail overlap).
                if tag == ("ep", 1):
                    if a1 is None:
                        a1 = attention(1)
                    next(a1, None)
            for _ in pa1:
                pass
            if a1 is None:
                a1 = attention(1)
            for _ in a1:
                pass

    _split_multi_waits(nc)
    return nc


def _get_nc():
    if "nc" not in _cache:
        _install_patches()
        _cache["nc"] = _build_nc()
    return _cache["nc"]


def _prep_in_maps(x, Wq, Wk, Wv):
    """Host-side input marshaling: shard x over batch, pre-transpose and
    cast to the layouts the kernel DMAs directly (pure layout/dtype work --
    all matmul/softmax FLOPs stay on-chip)."""
    import ml_dtypes

    bf = ml_dtypes.bfloat16
    x = np.asarray(x, dtype=np.float32)
    xt = x.transpose(0, 2, 1).astype(bf).reshape(B, 3, 128, S)
    wq = np.asarray(Wq, dtype=np.float32).reshape(3, 128, E)
    wk = np.asarray(Wk, dtype=np.float32).reshape(3, 128, E)
    wqk = np.concatenate([wq, wk], axis=2).astype(bf)
    wv3 = np.asarray(Wv, dtype=np.float32).reshape(3, 128, E).astype(bf)
    return [
        {
            "x": np.ascontiguousarray(xt[i * BPC : (i + 1) * BPC]),
            "wqk": wqk,
            "wv": wv3,
        }
        for i in range(N_CORES)
    ]


def kernel(x, Wq, Wk, Wv):
    from concourse.bass_utils import run_bass_kernel_spmd

    nc = _get_nc()
    in_maps = _prep_in_maps(x, Wq, Wk, Wv)
    res = run_bass_kernel_spmd(nc, in_maps, list(range(N_CORES)))
    out = np.concatenate([res.results[i]["out"] for i in range(N_CORES)], axis=0)
    return out.astype(np.float32)
